# revision 1
# baseline (speedup 1.0000x reference)
"""EdgeDecoder kernel for 8 Trainium2 NeuronCores.

Math: out[e] = dot(x_src[i0[e]], w_src) + dot(x_dst[i1[e]], w_dst) + bias.
Rewritten as per-node scores s[n] = x_src[n]@w_src + bias, d[n] = x_dst[n]@w_dst,
then out[e] = s[i0[e]] + d[i1[e]].

Sharding (host policy): nodes are split into 8 equal banks of 12500; the
s-side workload sorts edges by i0 and assigns each edge to the core owning
i0 (likewise d-side by i1).  Within a core, sorted edges are packed into
tiles of <=128 edges spanning <=W consecutive nodes, so the device gathers
one W-float window per tile (indirect DMA, 128 windows/instruction) and
resolves each edge by a host-precomputed one-hot select (2 vector-engine
passes: multiply + reduce).  A second tiny
launch adds the two host-realigned halves (device does all arithmetic; the
host only permutes/unshards between launches).
"""

import numpy as np

N_NODES = 100000
HIDDEN = 128
N_EDGES = 2000000
N_CORES = 8
NS = N_NODES // N_CORES          # 12500 nodes per core
A_TILES = 98                     # phase-A node tiles (padded)
NSP = A_TILES * 128              # 12544 padded nodes per core
CH = 7                           # phase-A tiles per chunk (98 = 14*7)
W = 5                            # window width (nodes per tile span)
T_CAP = 2688                     # tile capacity per side per core (21*128)
GROUPS = T_CAP // 128            # 21 indirect-gather instructions per side
E_CAP = T_CAP * 128              # 344064 edge slots per side per core
TAB = NSP                        # local table length (12544)
E_OUT = 250112                   # launch-2 per-core edges (128*1954)

_CACHE = {}


def _mybir():
    import concourse.mybir as mybir
    return mybir


def _build_launch1(reps=1):
    from contextlib import ExitStack
    import concourse.bass as bass
    import concourse.bacc as bacc
    import concourse.tile as tile
    mybir = _mybir()
    f32 = mybir.dt.float32
    i32 = mybir.dt.int32

    nc = bacc.Bacc("TRN2", debug=False, num_devices=N_CORES)
    xs = nc.dram_tensor("xs", [NSP, HIDDEN], f32, kind="ExternalInput")
    xd = nc.dram_tensor("xd", [NSP, HIDDEN], f32, kind="ExternalInput")
    wsr = nc.dram_tensor("wsr", [128, HIDDEN], f32, kind="ExternalInput")
    wdr = nc.dram_tensor("wdr", [128, HIDDEN], f32, kind="ExternalInput")
    biasr = nc.dram_tensor("biasr", [128, 1], f32, kind="ExternalInput")
    ident = nc.dram_tensor("ident", [128, 128], f32, kind="ExternalInput")
    sbase = nc.dram_tensor("sbase", [128, GROUPS], i32, kind="ExternalInput")
    dbase = nc.dram_tensor("dbase", [128, GROUPS], i32, kind="ExternalInput")
    soh = nc.dram_tensor("soh", [T_CAP * 128, W], f32, kind="ExternalInput")
    doh = nc.dram_tensor("doh", [T_CAP * 128, W], f32, kind="ExternalInput")
    g0 = nc.dram_tensor("g0", [E_CAP], f32, kind="ExternalOutput")
    g1 = nc.dram_tensor("g1", [E_CAP], f32, kind="ExternalOutput")

    add = mybir.AluOpType.add
    mult = mybir.AluOpType.mult
    is_eq = mybir.AluOpType.is_equal

    with tile.TileContext(nc) as tc:
        with tc.tile_pool(name="const", bufs=1) as cp, \
             tc.tile_pool(name="xload", bufs=6) as xp, \
             tc.tile_pool(name="work", bufs=3) as wp, \
             tc.tile_pool(name="sel", bufs=4) as sp, \
             tc.tile_pool(name="psum", bufs=2, space="PSUM") as pp, \
             tc.tile_pool(name="dram", bufs=1, space="DRAM") as dp:

            w_s = cp.tile([128, HIDDEN], f32)
            w_d = cp.tile([128, HIDDEN], f32)
            bias_t = cp.tile([128, 1], f32)
            id_t = cp.tile([128, 128], f32)
            nc.sync.dma_start(out=w_s[:], in_=wsr.ap()[:, :])
            nc.sync.dma_start(out=w_d[:], in_=wdr.ap()[:, :])
            nc.sync.dma_start(out=bias_t[:], in_=biasr.ap()[:, :])
            nc.sync.dma_start(out=id_t[:], in_=ident.ap()[:, :])

            s_dram = dp.tile([TAB, 1], f32)
            d_dram = dp.tile([TAB, 1], f32)

            _loop = ExitStack()
            if reps > 1:
                _loop.enter_context(tc.For_i(0, reps, 1))

            def phase_a(x, w_t, use_bias, table, nm):
                s_sb = wp.tile([128, A_TILES], f32, name=f"ssb_{nm}", tag="ssb")
                for c0 in range(0, A_TILES, CH):
                    xt = xp.tile([128, CH, HIDDEN], f32,
                                 name=f"xt_{nm}{c0}", tag="xt")
                    nc.sync.dma_start(
                        out=xt[:],
                        in_=x.ap()[c0 * 128:(c0 + CH) * 128, :].rearrange(
                            "(t p) h -> p t h", p=128))
                    scr = wp.tile([128, CH, HIDDEN], f32,
                                  name=f"scr_{nm}{c0}", tag="scr")
                    nc.vector.tensor_tensor(
                        out=scr[:],
                        in0=xt[:],
                        in1=w_t[:].rearrange("p h -> p () h").to_broadcast(
                            [128, CH, HIDDEN]),
                        op=mult)
                    nc.vector.tensor_reduce(
                        out=s_sb[:, c0:c0 + CH], in_=scr[:],
                        axis=mybir.AxisListType.X, op=add)
                if use_bias:
                    nc.vector.tensor_scalar_add(
                        out=s_sb[:], in0=s_sb[:], scalar1=bias_t[:, :])
                ps = pp.tile([A_TILES, 128], f32, name=f"ps_{nm}", tag="ps")
                nc.tensor.transpose(out=ps[:], in_=s_sb[:], identity=id_t[:])
                sT = wp.tile([A_TILES, 128], f32, name=f"sT_{nm}", tag="sT")
                nc.vector.tensor_copy(out=sT[:], in_=ps[:])
                nc.sync.dma_start(
                    out=table[:, 0].rearrange("(a b) -> a b", b=128),
                    in_=sT[:])

            phase_a(xs, w_s, True, s_dram, "s")
            phase_a(xd, w_d, False, d_dram, "d")

            def phase_bc(bases, ohd, table, gout, nm):
                bt = cp.tile([128, GROUPS], i32, name=f"bt_{nm}")
                nc.sync.dma_start(out=bt[:], in_=bases.ap()[:, :])
                win = cp.tile([128, GROUPS * W], f32, name=f"win_{nm}")
                for j in range(GROUPS):
                    nc.gpsimd.indirect_dma_start(
                        out=win[:, j * W:(j + 1) * W],
                        out_offset=None,
                        in_=table[:, :],
                        in_offset=bass.IndirectOffsetOnAxis(
                            ap=bt[:, j:j + 1], axis=0))
                # process groups in quads to cut per-instruction
                # overheads (GROUPS=21: 5 quads + 1 single)
                blocks = [(j, 4) for j in range(0, GROUPS - 1, 4)]
                blocks.append((GROUPS - 1, 1))
                for j, g in blocks:
                    oh = sp.tile([128, 4, 128 * W], f32, name=f"oh_{nm}{j}",
                                 tag="oh")
                    nc.sync.dma_start(
                        out=oh[:, :g, :],
                        in_=ohd.ap()[j * 16384:(j + g) * 16384, :].rearrange(
                            "(g p e) w -> p g (e w)", g=g, p=128))
                    pr = sp.tile([128, 4, 128, W], f32, name=f"pr_{nm}{j}",
                                 tag="pr")
                    nc.vector.tensor_tensor(
                        out=pr[:, :g, :, :],
                        in0=oh[:, :g, :].rearrange(
                            "p g (e w) -> p g e w", w=W),
                        in1=win[:, j * W:(j + g) * W].rearrange(
                            "p (g w) -> p g () w", g=g).to_broadcast(
                            [128, g, 128, W]),
                        op=mult)
                    ot = sp.tile([128, 4, 128], f32, name=f"ot_{nm}{j}",
                                 tag="ot")
                    nc.vector.tensor_reduce(
                        out=ot[:, :g, :], in_=pr[:, :g, :, :],
                        axis=mybir.AxisListType.X, op=add)
                    nc.sync.dma_start(
                        out=gout.ap()[j * 16384:(j + g) * 16384].rearrange(
                            "(g p e) -> p g e", g=g, p=128),
                        in_=ot[:, :g, :])

            phase_bc(sbase, soh, s_dram, g0, "s")
            phase_bc(dbase, doh, d_dram, g1, "d")
            _loop.close()

    nc.compile()
    return nc


def _build_launch2(reps=1):
    from contextlib import ExitStack
    import concourse.bacc as bacc
    import concourse.tile as tile
    mybir = _mybir()
    f32 = mybir.dt.float32
    COLS = E_OUT // 128  # 1954

    nc = bacc.Bacc("TRN2", debug=False, num_devices=N_CORES)
    a0 = nc.dram_tensor("a0", [128, COLS], f32, kind="ExternalInput")
    a1 = nc.dram_tensor("a1", [128, COLS], f32, kind="ExternalInput")
    o = nc.dram_tensor("o", [128, COLS], f32, kind="ExternalOutput")
    with tile.TileContext(nc) as tc:
        with tc.tile_pool(name="io", bufs=3) as io:
            _loop = ExitStack()
            if reps > 1:
                _loop.enter_context(tc.For_i(0, reps, 1))
            step = 512
            for c0 in range(0, COLS, step):
                c1 = min(c0 + step, COLS)
                t0 = io.tile([128, c1 - c0], f32, name=f"t0_{c0}", tag="t0")
                t1 = io.tile([128, c1 - c0], f32, name=f"t1_{c0}", tag="t1")
                to = io.tile([128, c1 - c0], f32, name=f"to_{c0}", tag="to")
                nc.sync.dma_start(out=t0[:], in_=a0.ap()[:, c0:c1])
                nc.sync.dma_start(out=t1[:], in_=a1.ap()[:, c0:c1])
                nc.vector.tensor_tensor(out=to[:], in0=t0[:], in1=t1[:],
                                        op=mybir.AluOpType.add)
                nc.sync.dma_start(out=o.ap()[:, c0:c1], in_=to[:])
            _loop.close()
    nc.compile()
    return nc


def _prep_side(iarr):
    """Sort edges by endpoint, shard by owning core, pack into W-span tiles.

    Returns bases [CORES,128,GROUPS] i32, oh [CORES,T_CAP*128,W] f32
    (one-hot of the within-window offset per edge slot), pos [E] i64
    (slot of edge e in the concatenated per-core g outputs)."""
    E = iarr.shape[0]
    order = np.argsort(iarr, kind="stable")
    srt = iarr[order]
    bases = np.zeros((N_CORES, 128, GROUPS), np.int32)
    oh = np.zeros((N_CORES, T_CAP * 128, W), np.float32)
    pos = np.empty(E, np.int64)
    for c in range(N_CORES):
        a = np.searchsorted(srt, c * NS, "left")
        b = np.searchsorted(srt, (c + 1) * NS, "left")
        li = srt[a:b] - c * NS
        eo = order[a:b]
        n = len(li)
        t = 0
        p = 0
        while p < n:
            base = int(li[p])
            end = min(p + 128, int(np.searchsorted(li, base + W, "left")))
            cnt = end - p
            bases[c, t % 128, t // 128] = base
            oh[c, t * 128 + np.arange(cnt), li[p:end] - base] = 1.0
            pos[eo[p:end]] = c * E_CAP + t * 128 + np.arange(cnt)
            t += 1
            p = end
        if t > T_CAP:
            raise RuntimeError(f"tile capacity exceeded: {t} > {T_CAP}")
    return bases, oh, pos


def _run_with_retry(nc, in_maps, attempts=3):
    """The axon-tunneled devices occasionally report a transient
    NRT_EXEC_UNIT_UNRECOVERABLE; a spaced retry usually succeeds."""
    import time
    from concourse import bass_utils
    last = None
    for k in range(attempts):
        try:
            return bass_utils.run_bass_kernel_spmd(
                nc, in_maps, core_ids=list(range(N_CORES)))
        except Exception as e:  # noqa: BLE001 - device transient
            last = e
            time.sleep(3.0 * (k + 1))
    raise last


def kernel(x_src, x_dst, edge_label_index, weight, bias):
    from concourse import bass_utils

    x_src = np.ascontiguousarray(np.asarray(x_src, dtype=np.float32))
    x_dst = np.ascontiguousarray(np.asarray(x_dst, dtype=np.float32))
    idx = np.asarray(edge_label_index)
    i0 = idx[0].astype(np.int64)
    i1 = idx[1].astype(np.int64)
    wgt = np.asarray(weight, dtype=np.float32)
    b = np.asarray(bias, dtype=np.float32)

    if "l1" not in _CACHE:
        _CACHE["l1"] = _build_launch1()
    if "l2" not in _CACHE:
        _CACHE["l2"] = _build_launch2()
    nc1, nc2 = _CACHE["l1"], _CACHE["l2"]

    sb, so, pos0 = _prep_side(i0)
    db, do, pos1 = _prep_side(i1)

    wsr = np.ascontiguousarray(np.broadcast_to(wgt[0, :HIDDEN], (128, HIDDEN)))
    wdr = np.ascontiguousarray(np.broadcast_to(wgt[0, HIDDEN:], (128, HIDDEN)))
    biasr = np.full((128, 1), b[0], np.float32)
    ident = np.eye(128, dtype=np.float32)

    pad = np.zeros((NSP - NS, HIDDEN), np.float32)
    in_maps1 = []
    for c in range(N_CORES):
        in_maps1.append({
            "xs": np.concatenate([x_src[c * NS:(c + 1) * NS], pad]),
            "xd": np.concatenate([x_dst[c * NS:(c + 1) * NS], pad]),
            "wsr": wsr, "wdr": wdr, "biasr": biasr,
            "ident": ident,
            "sbase": sb[c], "dbase": db[c],
            "soh": so[c], "doh": do[c],
        })
    res1 = _run_with_retry(nc1, in_maps1)
    G0 = np.concatenate([res1.results[c]["g0"] for c in range(N_CORES)])
    G1 = np.concatenate([res1.results[c]["g1"] for c in range(N_CORES)])

    a0 = np.zeros(N_CORES * E_OUT, np.float32)
    a1 = np.zeros(N_CORES * E_OUT, np.float32)
    per = N_EDGES // N_CORES  # 250000 real edges per launch-2 core
    for c in range(N_CORES):
        e0, e1 = c * per, (c + 1) * per
        a0[c * E_OUT:c * E_OUT + per] = G0[pos0[e0:e1]]
        a1[c * E_OUT:c * E_OUT + per] = G1[pos1[e0:e1]]

    in_maps2 = [{
        "a0": a0[c * E_OUT:(c + 1) * E_OUT].reshape(128, E_OUT // 128),
        "a1": a1[c * E_OUT:(c + 1) * E_OUT].reshape(128, E_OUT // 128),
    } for c in range(N_CORES)]
    res2 = _run_with_retry(nc2, in_maps2)

    out = np.empty(N_EDGES, np.float32)
    for c in range(N_CORES):
        out[c * per:(c + 1) * per] = \
            res2.results[c]["o"].reshape(-1)[:per]
    return out.reshape(N_EDGES, 1)



# revision 14
# speedup vs baseline: 4.4656x; 4.4656x over previous
"""EdgeDecoder kernel for 8 Trainium2 NeuronCores.

Math: out[e] = dot(x_src[i0[e]], w_src) + dot(x_dst[i1[e]], w_dst) + bias.
Rewritten as per-node scores s[n] = x_src[n]@w_src + bias, d[n] = x_dst[n]@w_dst,
then out[e] = s[i0[e]] + d[i1[e]].

Device pipeline (launch 1, per core, per side):
  - Host packs each core's ~250k edges into 128*G tiles of F=40 slots; a
    tile holds edges of at most TWO nodes (free pairing, ~98% fill).
  - Host stages x per tile-node as bf16 [h=128, half, g, m=128]: the two
    nodes of tile (p, g) sit in matmul chunk (half=0, g) and (half=1, g),
    column p.  Phase A is pure PE: 2*G chunk-stationary matmuls
    (lhsT = x^T chunk, rhs = w as [128,1]) land both per-tile endpoint
    scores in PSUM [128, 2, G] - already in window order, so there is no
    score table, no DRAM round-trip and no indirect gather at all.
  - One ACT copy (f32->bf16) -> W [128, 2, G]; dif = W1-W0 (DVE).
  - Per-edge value is a lerp  g = W0 + off*(W1-W0)  with host-shipped
    off in {0,1} (bf16 [128, F, G]): two bf16 DVE passes in 2x_1p mode.
Launch 2 adds the two host-realigned halves (device arithmetic only; the
host only permutes/casts between launches).
"""

import numpy as np
import ml_dtypes

BF16 = ml_dtypes.bfloat16

N_NODES = 100000
HIDDEN = 128
N_EDGES = 2000000
N_CORES = 8
NS = N_NODES // N_CORES         # 12500 nodes per core
F = 40                          # edge slots per tile
G = 51                          # tiles per partition (data needs 50)
S = 128 * F * G                 # 261120 edge slots per side per core
PER = N_EDGES // N_CORES        # 250000 edges per launch-2 core
COLS = (PER + 127) // 128       # 1954
E_OUT = COLS * 128              # 250112 padded launch-2 edges per core

_CACHE = {}


def _mybir():
    import concourse.mybir as mybir
    return mybir


def _build_launch1(reps=1):
    from contextlib import ExitStack
    import concourse.bacc as bacc
    import concourse.tile as tile
    mybir = _mybir()
    f32 = mybir.dt.float32
    bf16 = mybir.dt.bfloat16

    nc = bacc.Bacc("TRN2", debug=False, num_devices=N_CORES)
    xs = nc.dram_tensor("xs", [128, 2, G, 128], bf16, kind="ExternalInput")
    xd = nc.dram_tensor("xd", [128, 2, G, 128], bf16, kind="ExternalInput")
    wv = nc.dram_tensor("wv", [128, 2], bf16, kind="ExternalInput")
    biasr = nc.dram_tensor("biasr", [128, 1], f32, kind="ExternalInput")
    soff = nc.dram_tensor("soff", [128, F, G], bf16, kind="ExternalInput")
    doff = nc.dram_tensor("doff", [128, F, G], bf16, kind="ExternalInput")
    g0 = nc.dram_tensor("g0", [128, F, G], bf16, kind="ExternalOutput")
    g1 = nc.dram_tensor("g1", [128, F, G], bf16, kind="ExternalOutput")

    add = mybir.AluOpType.add
    mult = mybir.AluOpType.mult
    sub = mybir.AluOpType.subtract
    XCH = 17  # g-columns per x-load DMA (3 loads per half)

    with tile.TileContext(nc) as tc:
        with tc.tile_pool(name="const", bufs=1) as cp, \
             tc.tile_pool(name="xload", bufs=4) as xp, \
             tc.tile_pool(name="work", bufs=2) as wp, \
             tc.tile_pool(name="psum", bufs=2, space="PSUM") as pp:

            wv_t = cp.tile([128, 2], bf16)
            nc.sync.dma_start(out=wv_t[:], in_=wv.ap()[:, :])
            bias_t = cp.tile([128, 1], f32, name="bias_t")
            nc.sync.dma_start(out=bias_t[:], in_=biasr.ap()[:, :])

            _loop = ExitStack()
            if reps > 1:
                _loop.enter_context(
                    tc.For_i(0, reps, 1,
                             hint_engines=(mybir.EngineType.PE,)))

            def side(x, wcol, offs, gout, nm, use_bias):
                # phase A: per-tile endpoint scores, in window order
                ps = pp.tile([128, 2, G], f32, name=f"ps_{nm}", tag="ps")
                for half in range(2):
                    for c0 in range(0, G, XCH):
                        c1 = min(c0 + XCH, G)
                        xt = xp.tile([128, XCH, 128], bf16,
                                     name=f"xt_{nm}{half}{c0}", tag="xt")
                        nc.sync.dma_start(
                            out=xt[:, :c1 - c0, :],
                            in_=x.ap()[:, half, c0:c1, :])
                        for j in range(c1 - c0):
                            nc.tensor.matmul(
                                ps[:, half, c0 + j:c0 + j + 1],
                                xt[:, j, :],
                                wv_t[:, wcol:wcol + 1])
                w = wp.tile([128, 2, G], bf16, name=f"w_{nm}", tag="w")
                nc.scalar.activation(
                    out=w[:], in_=ps[:],
                    func=mybir.ActivationFunctionType.Copy)
                if use_bias:
                    nc.vector.tensor_scalar_add(
                        out=w[:], in0=w[:], scalar1=bias_t[:, :])
                dif = wp.tile([128, G], bf16, name=f"dif_{nm}", tag="dif")
                nc.vector.tensor_tensor(
                    out=dif[:], in0=w[:, 1, :], in1=w[:, 0, :], op=sub)

                # phase B: lerp select
                offt = wp.tile([128, F, G], bf16, name=f"offt_{nm}",
                               tag="offt")
                nc.scalar.dma_start(out=offt[:], in_=offs.ap()[:, :, :])
                prod = wp.tile([128, F, G], bf16, name=f"prod_{nm}",
                               tag="prod")
                nc.vector.tensor_tensor(
                    out=prod[:],
                    in0=offt[:],
                    in1=dif[:].rearrange("p g -> p () g").to_broadcast(
                        [128, F, G]),
                    op=mult)
                gt = wp.tile([128, F, G], bf16, name=f"gt_{nm}", tag="gt")
                nc.vector.tensor_tensor(
                    out=gt[:],
                    in0=prod[:],
                    in1=w[:, 0, :].rearrange("p g -> p () g").to_broadcast(
                        [128, F, G]),
                    op=add)
                nc.scalar.dma_start(out=gout.ap()[:, :, :], in_=gt[:])

            side(xs, 0, soff, g0, "s", True)
            side(xd, 1, doff, g1, "d", False)
            _loop.close()

    nc.compile()
    return nc


def _build_launch2(reps=1):
    from contextlib import ExitStack
    import concourse.bacc as bacc
    import concourse.tile as tile
    mybir = _mybir()
    bf16 = mybir.dt.bfloat16

    nc = bacc.Bacc("TRN2", debug=False, num_devices=N_CORES)
    a0 = nc.dram_tensor("a0", [128, COLS], bf16, kind="ExternalInput")
    a1 = nc.dram_tensor("a1", [128, COLS], bf16, kind="ExternalInput")
    o = nc.dram_tensor("o", [128, COLS], bf16, kind="ExternalOutput")
    with tile.TileContext(nc) as tc:
        with tc.tile_pool(name="io", bufs=3) as io:
            _loop = ExitStack()
            if reps > 1:
                _loop.enter_context(tc.For_i(0, reps, 1))
            step = 977
            for c0 in range(0, COLS, step):
                c1 = min(c0 + step, COLS)
                t0 = io.tile([128, c1 - c0], bf16, name=f"t0_{c0}", tag="t0")
                t1 = io.tile([128, c1 - c0], bf16, name=f"t1_{c0}", tag="t1")
                to = io.tile([128, c1 - c0], bf16, name=f"to_{c0}", tag="to")
                nc.sync.dma_start(out=t0[:], in_=a0.ap()[:, c0:c1])
                nc.sync.dma_start(out=t1[:], in_=a1.ap()[:, c0:c1])
                nc.vector.tensor_tensor(out=to[:], in0=t0[:], in1=t1[:],
                                        op=mybir.AluOpType.add)
                nc.sync.dma_start(out=o.ap()[:, c0:c1], in_=to[:])
            _loop.close()
    nc.compile()
    return nc


def _prep_side(iarr):
    """Per-core: pack edges into F-slot tiles of at most 2 nodes each
    (big nodes split into full tiles; leftovers two-pointer paired).

    Returns nodesAB [CORES,2,128,G] i64 (local node per tile half),
    off [CORES,128,F,G] bf16, pos [E] i64 (slot of edge e in its core's
    g output, slot index = p*(F*G) + f*G + g)."""
    E = iarr.shape[0]
    nodesAB = np.zeros((N_CORES, 2, 128, G), np.int64)
    off = np.zeros((N_CORES, 128, F, G), BF16)
    pos = np.empty(E, np.int64)
    one = BF16(1.0)
    for c in range(N_CORES):
        sel = np.nonzero((iarr >= c * NS) & (iarr < (c + 1) * NS))[0]
        li = iarr[sel] - c * NS
        so = np.argsort(li, kind="stable")
        sli = li[so]
        sedge = sel[so]
        counts = np.bincount(sli, minlength=NS)
        starts = np.concatenate([[0], np.cumsum(counts)])
        # tiles: (nodeA, sliceA, nodeB, sliceB)
        tiles = []
        rem = []  # (count, node, start_index)
        for n in range(NS):
            cnt = int(counts[n])
            st = int(starts[n])
            nfull = cnt // F
            for k in range(nfull):
                tiles.append((n, st + k * F, F, n, 0, 0))
            r = cnt % F
            if r:
                rem.append((r, n, st + nfull * F))
        rem.sort()
        i, j = 0, len(rem) - 1
        while i <= j:
            ra, na, sa = rem[j]
            if i < j and ra + rem[i][0] <= F:
                rb, nb, sbst = rem[i]
                tiles.append((na, sa, ra, nb, sbst, rb))
                i += 1
                j -= 1
            else:
                tiles.append((na, sa, ra, na, 0, 0))
                j -= 1
        if len(tiles) > 128 * G:
            raise RuntimeError(
                f"tile capacity exceeded on core {c}: {len(tiles)}")
        for t, (na, sa, ca, nb, sbst, cb) in enumerate(tiles):
            p, g = t % 128, t // 128
            nodesAB[c, 0, p, g] = na
            nodesAB[c, 1, p, g] = nb
            eA = sedge[sa:sa + ca]
            pos[eA] = p * (F * G) + np.arange(ca) * G + g
            if cb:
                eB = sedge[sbst:sbst + cb]
                pos[eB] = p * (F * G) + (ca + np.arange(cb)) * G + g
                off[c, p, ca:ca + cb, g] = one
    return nodesAB, off, pos


def _stage_x(x, nodes):
    """x slice [NS, H] f32 -> bf16 [h=128, 2, G, m=128]: chunk (half, g)
    column m holds x of local node nodes[half, m, g]."""
    xb = x.astype(BF16)                       # [NS, H]
    sel = xb[nodes.reshape(2, 128, G)]        # [2, 128m, G, H]
    return np.ascontiguousarray(sel.transpose(3, 0, 2, 1))


def _run_with_retry(nc, in_maps, attempts=3):
    """The axon-tunneled devices occasionally report a transient
    NRT_EXEC_UNIT_UNRECOVERABLE; a spaced retry usually succeeds."""
    import time
    from concourse import bass_utils
    last = None
    for k in range(attempts):
        try:
            return bass_utils.run_bass_kernel_spmd(
                nc, in_maps, core_ids=list(range(N_CORES)))
        except Exception as e:  # noqa: BLE001 - device transient
            last = e
            time.sleep(3.0 * (k + 1))
    raise last


def kernel(x_src, x_dst, edge_label_index, weight, bias):
    x_src = np.ascontiguousarray(np.asarray(x_src, dtype=np.float32))
    x_dst = np.ascontiguousarray(np.asarray(x_dst, dtype=np.float32))
    idx = np.asarray(edge_label_index)
    i0 = idx[0].astype(np.int64)
    i1 = idx[1].astype(np.int64)
    wgt = np.asarray(weight, dtype=np.float32)
    b = np.asarray(bias, dtype=np.float32)

    if "l1" not in _CACHE:
        _CACHE["l1"] = _build_launch1()
    if "l2" not in _CACHE:
        _CACHE["l2"] = _build_launch2()
    nc1, nc2 = _CACHE["l1"], _CACHE["l2"]

    nodes0, so, pos0 = _prep_side(i0)
    nodes1, do, pos1 = _prep_side(i1)

    # w staged on partitions (K = h), one column per side
    wv = np.zeros((128, 2), BF16)
    wv[:, 0] = wgt[0, :HIDDEN].astype(BF16)
    wv[:, 1] = wgt[0, HIDDEN:].astype(BF16)

    in_maps1 = []
    for c in range(N_CORES):
        in_maps1.append({
            "xs": _stage_x(x_src[c * NS:(c + 1) * NS], nodes0[c]),
            "xd": _stage_x(x_dst[c * NS:(c + 1) * NS], nodes1[c]),
            "wv": wv,
            "biasr": np.full((128, 1), b[0], np.float32),
            "soff": so[c], "doff": do[c],
        })
    res1 = _run_with_retry(nc1, in_maps1)
    G0 = np.concatenate(
        [res1.results[c]["g0"].reshape(-1) for c in range(N_CORES)])
    G1 = np.concatenate(
        [res1.results[c]["g1"].reshape(-1) for c in range(N_CORES)])

    # realign halves to edge order (host permutation only)
    a0 = np.zeros(N_CORES * E_OUT, BF16)
    a1 = np.zeros(N_CORES * E_OUT, BF16)
    v0 = G0[(i0 // NS) * S + pos0]
    v1 = G1[(i1 // NS) * S + pos1]
    for c in range(N_CORES):
        e0, e1 = c * PER, (c + 1) * PER
        a0[c * E_OUT:c * E_OUT + PER] = v0[e0:e1]
        a1[c * E_OUT:c * E_OUT + PER] = v1[e0:e1]

    in_maps2 = [{
        "a0": a0[c * E_OUT:(c + 1) * E_OUT].reshape(128, COLS),
        "a1": a1[c * E_OUT:(c + 1) * E_OUT].reshape(128, COLS),
    } for c in range(N_CORES)]
    res2 = _run_with_retry(nc2, in_maps2)

    out = np.empty(N_EDGES, np.float32)
    for c in range(N_CORES):
        out[c * PER:(c + 1) * PER] = \
            res2.results[c]["o"].reshape(-1)[:PER].astype(np.float32)
    return out.reshape(N_EDGES, 1)
